# revision 5
# baseline (speedup 1.0000x reference)
"""AlignmentAttention Trainium2 kernel (8 NeuronCores, pure data parallel over B).

Math: reference computes
    key    = einsum("nbsr,er->nbse", kv, Wk) + bk
    scores = einsum("bte,nbse->nbts", q, key) + mask
    out    = softmax(scores) @ kv
Because softmax is invariant to per-row constants, the bias term q@bk cancels,
and q @ (kv@Wk^T)^T == (q@Wk) @ kv^T.  So we project the *query* once per batch
element (qproj = q@Wk, shared across all N candidates) instead of projecting
keys N times, and drop bk entirely.

Sharding: one batch element b per core (B=8 == n_cores).  Per core:
    qprojT = (q_b @ Wk)^T              64 matmuls   (fp16 operands, f32 psum)
    per candidate n:
        scores  = qproj @ kvT_nb        32 matmuls
        softmax: DVE mask-add + rowmax, ACT fused exp+rowsum -> fp16 attn
        attn^T via fp16 PE transpose (1 cyc/row) into fp16 psum
        out_nb  = attn @ kv_nb          32 matmuls, 1/rowsum fused into the
                  fp16 eviction; host upcasts the fp16 result to f32

Perf notes vs v1:
  - PE p-state ramp (0.65->1.2->2.4 GHz after ~3us sustained activity): keep
    the PE streaming dummy fp16 transposes through the DMA prologue so real
    matmuls start at full clock.
  - wk is shipped host-side in a column-block layout so qproj's first psum
    only needs qT + 1/8th of wk -> first real matmul at ~1.5us instead of ~8.
  - attn transpose in fp16 (1 cyc/row vs 2 for f32) into fp16 psum.
  - transpose+out matmuls emitted per-ti so a late softmax(ti=3) can't block
    ready out-matmuls of ti=0..2 in the PE queue.
  - input loads on sync queue, wk blocks on vector queue, out stores on
    gpsimd queue; out is fp16 (halves the tail-store bytes).
"""
import contextlib
import os
import sys

import numpy as np

_TRN_REPO = "/opt/trn_rl_repo"
if _TRN_REPO not in sys.path and os.path.isdir(_TRN_REPO):
    sys.path.insert(0, _TRN_REPO)

# jax on the native neuron backend crashes; the axon PJRT proxy path needs the
# default platform selection.
if os.environ.get("JAX_PLATFORMS") == "cpu":
    os.environ["JAX_PLATFORMS"] = ""

import concourse.bacc as bacc
import concourse.tile as tile
from concourse import mybir
from concourse.masks import make_identity
from concourse.bass_utils import run_bass_kernel_spmd

F32 = mybir.dt.float32
F16 = mybir.dt.float16

N_CAND, B, T, S, E, R = 4, 8, 512, 512, 1024, 1024
TT, ST, ET, RT = T // 128, S // 128, E // 128, R // 128

_NC_CACHE = []


def build_nc():
    nc = bacc.Bacc(None, target_bir_lowering=False)
    qT = nc.declare_dram_parameter("qT", [E, T], F16, isOutput=False)
    kv = nc.declare_dram_parameter("kv", [N_CAND, S, R], F16, isOutput=False)
    kvT = nc.declare_dram_parameter("kvT", [N_CAND, R, S], F16, isOutput=False)
    mask = nc.declare_dram_parameter("mask", [T, S], F32, isOutput=False)
    # wk shipped column-block-major: wkcb[rb, p, eh*128+rl] = Wk[eh*128+p, rb*128+rl]
    wkcb = nc.declare_dram_parameter("wkcb", [RT, 128, E], F16, isOutput=False)
    out = nc.declare_dram_parameter("out", [N_CAND, T, R], F16, isOutput=True)

    with contextlib.ExitStack() as ctx:
        tc = ctx.enter_context(tile.TileContext(nc))
        singles = ctx.enter_context(tc.tile_pool(name="singles", bufs=1))
        kvpool = ctx.enter_context(tc.tile_pool(name="kvpool", bufs=2))
        kvtpool = ctx.enter_context(tc.tile_pool(name="kvtpool", bufs=2))
        scorepool = ctx.enter_context(tc.tile_pool(name="scorepool", bufs=3))
        attnpool = ctx.enter_context(tc.tile_pool(name="attnpool", bufs=3))
        attntpool = ctx.enter_context(tc.tile_pool(name="attntpool", bufs=2))
        outpool = ctx.enter_context(tc.tile_pool(name="outpool", bufs=3))
        smalls = ctx.enter_context(tc.tile_pool(name="smalls", bufs=10))
        psW = ctx.enter_context(tc.tile_pool(name="psW", bufs=1, space="PSUM"))
        psT = ctx.enter_context(tc.tile_pool(name="psT", bufs=2, space="PSUM"))
        psmm = ctx.enter_context(tc.tile_pool(name="psmm", bufs=5, space="PSUM"))

        ident16 = singles.tile([128, 128], F16)
        make_identity(nc, ident16)

        # Keep the PE busy through the DMA prologue so the p-state ramp
        # (full clock after ~3us of sustained activity) completes before the
        # first real matmul.  fp16 transposes: ~128 cycles each.
        wp = psW.tile([128, 512], F16, tag="warm")
        for k in range(40):
            nc.tensor.transpose(wp[:, (k % 4) * 128:(k % 4 + 1) * 128],
                                ident16, ident16)

        # persistent SBUF
        qT_sb = singles.tile([128, ET, T], F16)
        wk_sb = singles.tile([128, RT, E], F16)
        mask_sb = singles.tile([128, TT, S], F32)
        qprojT = singles.tile([128, RT, T], F16)

        # qT chunks first (every qproj psum needs all of qT), then the wk
        # column blocks stream one per psum tile.
        for e in range(ET):
            nc.sync.dma_start(out=qT_sb[:, e, :], in_=qT[e * 128:(e + 1) * 128, :])
        for rb in range(RT):
            nc.scalar.dma_start(out=wk_sb[:, rb, :], in_=wkcb[rb])

        # qprojT[rb*128+rl, t] = sum_e Wk[e, rb*128+rl] * qT[e, t]
        for rb in range(RT):
            p = psmm.tile([128, T], F32)
            for eh in range(ET):
                nc.tensor.matmul(p, wk_sb[:, rb, eh * 128:(eh + 1) * 128],
                                 qT_sb[:, eh, :],
                                 start=(eh == 0), stop=(eh == ET - 1))
            nc.scalar.copy(qprojT[:, rb, :], p)

        # mask is needed only at the first softmax; keep it off the critical
        # prologue path (after the first candidate's kv loads below).
        first_kv_loads = True

        for n in range(N_CAND):
            kvT_sb = kvtpool.tile([128, RT, S], F16)
            nc.sync.dma_start(out=kvT_sb,
                              in_=kvT[n].rearrange("(rh p) s -> p rh s", p=128))
            kv_sb = kvpool.tile([128, ST, R], F16)
            nc.sync.dma_start(out=kv_sb,
                              in_=kv[n].rearrange("(sh p) r -> p sh r", p=128))
            if first_kv_loads:
                nc.sync.dma_start(out=mask_sb,
                                  in_=mask.rearrange("(th p) s -> p th s", p=128))
                first_kv_loads = False

            # scores[t, s] = sum_r qproj[t, r] * kv[s, r]
            score_ps = []
            for ti in range(TT):
                p = psmm.tile([128, S], F32)
                for ri in range(RT):
                    nc.tensor.matmul(p, qprojT[:, ri, ti * 128:(ti + 1) * 128],
                                     kvT_sb[:, ri, :],
                                     start=(ri == 0), stop=(ri == RT - 1))
                score_ps.append(p)

            # softmax, unnormalized: attn_u = exp(scores + mask - rowmax) in
            # fp16; keep 1/rowsum for the eviction of the out matmul.
            attns = []
            recips = []
            for ti in range(TT):
                scores = scorepool.tile([128, S], F32)
                nc.vector.tensor_add(scores, score_ps[ti], mask_sb[:, ti, :])
                negmax = smalls.tile([128, 1], F32)
                nc.vector.tensor_reduce(negmax, scores, axis=mybir.AxisListType.X,
                                        op=mybir.AluOpType.max, negate=True)
                attn = attnpool.tile([128, S], F16)
                sumexp = smalls.tile([128, 1], F32)
                nc.scalar.activation(attn, scores, mybir.ActivationFunctionType.Exp,
                                     bias=negmax, scale=1.0, accum_out=sumexp)
                recip = smalls.tile([128, 1], F32)
                nc.vector.reciprocal(recip, sumexp)
                attns.append(attn)
                recips.append(recip)

            # per ti: fp16 PE transpose of attn, then immediately the out
            # matmuls for that ti (so a late softmax can't block ready work
            # behind it in the PE queue).
            attnT = attntpool.tile([128, ST, T], F16)
            for ti in range(TT):
                pT = psT.tile([128, 512], F16)
                for si in range(ST):
                    nc.tensor.transpose(pT[:, si * 128:(si + 1) * 128],
                                        attns[ti][:, si * 128:(si + 1) * 128],
                                        ident16)
                nc.vector.tensor_copy(
                    attnT[:, 0:ST, ti * 128:(ti + 1) * 128],
                    pT.rearrange("p (k j) -> p k j", k=ST))

                # out[t, r] = sum_s attn_u[t, s] kv[s, r]; normalize on eviction
                for rh in range(2):
                    p = psmm.tile([128, 512], F32)
                    for si in range(ST):
                        nc.tensor.matmul(p, attnT[:, si, ti * 128:(ti + 1) * 128],
                                         kv_sb[:, si, rh * 512:(rh + 1) * 512],
                                         start=(si == 0), stop=(si == ST - 1))
                    o = outpool.tile([128, 512], F16)
                    nc.scalar.mul(o, p, recips[ti])
                    nc.gpsimd.dma_start(
                        out=out[n, ti * 128:(ti + 1) * 128, rh * 512:(rh + 1) * 512],
                        in_=o)

    nc.compile()
    return nc


def make_in_maps(query, key_value_states, attention_mask, Wk):
    wkcb = np.ascontiguousarray(
        Wk.reshape(ET, 128, RT, 128).transpose(2, 1, 0, 3).reshape(RT, 128, E)
    ).astype(np.float16)
    in_maps = []
    for b in range(B):
        in_maps.append({
            "qT": np.ascontiguousarray(query[0, b].T).astype(np.float16),
            "kv": np.ascontiguousarray(key_value_states[:, b]).astype(np.float16),
            "kvT": np.ascontiguousarray(
                key_value_states[:, b].transpose(0, 2, 1)).astype(np.float16),
            "mask": np.ascontiguousarray(attention_mask[0, b]).astype(np.float32),
            "wkcb": wkcb,
        })
    return in_maps


def kernel(query, key_value_states, attention_mask, Wk, bk):
    query = np.asarray(query, dtype=np.float32)
    key_value_states = np.asarray(key_value_states, dtype=np.float32)
    attention_mask = np.asarray(attention_mask, dtype=np.float32)
    Wk = np.asarray(Wk, dtype=np.float32)
    del bk  # cancels inside the softmax (constant along the softmax axis)

    if not _NC_CACHE:
        _NC_CACHE.append(build_nc())
    nc = _NC_CACHE[0]

    in_maps = make_in_maps(query, key_value_states, attention_mask, Wk)
    res = run_bass_kernel_spmd(nc, in_maps, core_ids=list(range(B)))

    out = np.empty((N_CAND, B, T, R), dtype=np.float32)
    for b in range(B):
        out[:, b] = res.results[b]["out"].astype(np.float32)
    return out


# revision 6
# speedup vs baseline: 1.0812x; 1.0812x over previous
"""AlignmentAttention Trainium2 kernel (8 NeuronCores, pure data parallel over B).

Math: reference computes
    key    = einsum("nbsr,er->nbse", kv, Wk) + bk
    scores = einsum("bte,nbse->nbts", q, key) + mask
    out    = softmax(scores) @ kv
Because softmax is invariant to per-row constants, the bias term q@bk cancels,
and q @ (kv@Wk^T)^T == (q@Wk) @ kv^T.  So we project the *query* once per batch
element (qproj = q@Wk, shared across all N candidates) instead of projecting
keys N times, and drop bk entirely.

Sharding: one batch element b per core (B=8 == n_cores).  Per core:
    qprojT = (q_b @ Wk)^T              64 matmuls   (fp16 operands, f32 psum)
    per candidate n:
        scores  = qproj @ kvT_nb        32 matmuls
        softmax: DVE mask-add + rowmax, ACT fused exp+rowsum -> fp16 attn
        attn^T via fp16 PE transpose (1 cyc/row) into fp16 psum
        out_nb  = attn @ kv_nb          32 matmuls, 1/rowsum fused into the
                  fp16 eviction; host upcasts the fp16 result to f32

Perf notes vs v1:
  - PE p-state ramp (0.65->1.2->2.4 GHz after ~3us sustained activity): keep
    the PE streaming dummy fp16 transposes through the DMA prologue so real
    matmuls start at full clock.
  - wk is shipped host-side in a column-block layout so qproj's first psum
    only needs qT + 1/8th of wk -> first real matmul at ~1.5us instead of ~8.
  - attn transpose in fp16 (1 cyc/row vs 2 for f32) into fp16 psum.
  - transpose+out matmuls emitted per-ti so a late softmax(ti=3) can't block
    ready out-matmuls of ti=0..2 in the PE queue.
  - input loads on sync queue, wk blocks on vector queue, out stores on
    gpsimd queue; out is fp16 (halves the tail-store bytes).
"""
import contextlib
import os
import sys

import numpy as np

_TRN_REPO = "/opt/trn_rl_repo"
if _TRN_REPO not in sys.path and os.path.isdir(_TRN_REPO):
    sys.path.insert(0, _TRN_REPO)

# jax on the native neuron backend crashes; the axon PJRT proxy path needs the
# default platform selection.
if os.environ.get("JAX_PLATFORMS") == "cpu":
    os.environ["JAX_PLATFORMS"] = ""

import concourse.bacc as bacc
import concourse.tile as tile
from concourse import mybir
from concourse.masks import make_identity
from concourse.bass_utils import run_bass_kernel_spmd

F32 = mybir.dt.float32
F16 = mybir.dt.float16

N_CAND, B, T, S, E, R = 4, 8, 512, 512, 1024, 1024
TT, ST, ET, RT = T // 128, S // 128, E // 128, R // 128

_NC_CACHE = []


def build_nc():
    nc = bacc.Bacc(None, target_bir_lowering=False)
    qT = nc.declare_dram_parameter("qT", [E, T], F16, isOutput=False)
    kv = nc.declare_dram_parameter("kv", [N_CAND, S, R], F16, isOutput=False)
    kvT = nc.declare_dram_parameter("kvT", [N_CAND, R, S], F16, isOutput=False)
    mask = nc.declare_dram_parameter("mask", [T, S], F32, isOutput=False)
    # wk shipped column-block-major: wkcb[rb, p, eh*128+rl] = Wk[eh*128+p, rb*128+rl]
    wkcb = nc.declare_dram_parameter("wkcb", [RT, 128, E], F16, isOutput=False)
    out = nc.declare_dram_parameter("out", [N_CAND, T, R], F16, isOutput=True)

    with contextlib.ExitStack() as ctx:
        tc = ctx.enter_context(tile.TileContext(nc))
        singles = ctx.enter_context(tc.tile_pool(name="singles", bufs=1))
        kvpool = ctx.enter_context(tc.tile_pool(name="kvpool", bufs=2))
        kvtpool = ctx.enter_context(tc.tile_pool(name="kvtpool", bufs=2))
        scorepool = ctx.enter_context(tc.tile_pool(name="scorepool", bufs=3))
        attnpool = ctx.enter_context(tc.tile_pool(name="attnpool", bufs=3))
        attntpool = ctx.enter_context(tc.tile_pool(name="attntpool", bufs=2))
        outpool = ctx.enter_context(tc.tile_pool(name="outpool", bufs=3))
        smalls = ctx.enter_context(tc.tile_pool(name="smalls", bufs=10))
        psT = ctx.enter_context(tc.tile_pool(name="psT", bufs=2, space="PSUM"))
        psmm = ctx.enter_context(tc.tile_pool(name="psmm", bufs=6, space="PSUM"))

        ident16 = singles.tile([128, 128], F16)
        make_identity(nc, ident16)

        # persistent SBUF
        qT_sb = singles.tile([128, ET, T], F16)
        wk_sb = singles.tile([128, RT, E], F16)
        mask_sb = singles.tile([128, TT, S], F32)
        qprojT = singles.tile([128, RT, T], F16)

        # qT chunks first (every qproj psum needs all of qT), then the wk
        # column blocks stream one per psum tile.
        for e in range(ET):
            nc.sync.dma_start(out=qT_sb[:, e, :], in_=qT[e * 128:(e + 1) * 128, :])
        for rb in range(RT):
            nc.scalar.dma_start(out=wk_sb[:, rb, :], in_=wkcb[rb])

        # qprojT[rb*128+rl, t] = sum_e Wk[e, rb*128+rl] * qT[e, t]
        for rb in range(RT):
            p = psmm.tile([128, T], F32)
            for eh in range(ET):
                nc.tensor.matmul(p, wk_sb[:, rb, eh * 128:(eh + 1) * 128],
                                 qT_sb[:, eh, :],
                                 start=(eh == 0), stop=(eh == ET - 1))
            nc.scalar.copy(qprojT[:, rb, :], p)

        # mask is needed only at the first softmax; keep it off the critical
        # prologue path (after the first candidate's kv loads below).
        first_kv_loads = True

        for n in range(N_CAND):
            kvT_sb = kvtpool.tile([128, RT, S], F16)
            nc.sync.dma_start(out=kvT_sb,
                              in_=kvT[n].rearrange("(rh p) s -> p rh s", p=128))
            kv_sb = kvpool.tile([128, ST, R], F16)
            nc.sync.dma_start(out=kv_sb,
                              in_=kv[n].rearrange("(sh p) r -> p sh r", p=128))
            if first_kv_loads:
                nc.sync.dma_start(out=mask_sb,
                                  in_=mask.rearrange("(th p) s -> p th s", p=128))
                first_kv_loads = False

            # scores[t, s] = sum_r qproj[t, r] * kv[s, r]
            score_ps = []
            for ti in range(TT):
                p = psmm.tile([128, S], F32)
                for ri in range(RT):
                    nc.tensor.matmul(p, qprojT[:, ri, ti * 128:(ti + 1) * 128],
                                     kvT_sb[:, ri, :],
                                     start=(ri == 0), stop=(ri == RT - 1))
                score_ps.append(p)

            # softmax, unnormalized: attn_u = exp(scores + mask - rowmax) in
            # fp16; keep 1/rowsum for the eviction of the out matmul.
            attns = []
            recips = []
            for ti in range(TT):
                scores = scorepool.tile([128, S], F32)
                nc.vector.tensor_add(scores, score_ps[ti], mask_sb[:, ti, :])
                negmax = smalls.tile([128, 1], F32)
                nc.vector.tensor_reduce(negmax, scores, axis=mybir.AxisListType.X,
                                        op=mybir.AluOpType.max, negate=True)
                attn = attnpool.tile([128, S], F16)
                sumexp = smalls.tile([128, 1], F32)
                nc.scalar.activation(attn, scores, mybir.ActivationFunctionType.Exp,
                                     bias=negmax, scale=1.0, accum_out=sumexp)
                recip = smalls.tile([128, 1], F32)
                nc.vector.reciprocal(recip, sumexp)
                attns.append(attn)
                recips.append(recip)

            # per ti: fp16 PE transpose of attn, then immediately the out
            # matmuls for that ti (so a late softmax can't block ready work
            # behind it in the PE queue).
            attnT = attntpool.tile([128, ST, T], F16)
            for ti in range(TT):
                pT = psT.tile([128, 512], F16)
                for si in range(ST):
                    nc.tensor.transpose(pT[:, si * 128:(si + 1) * 128],
                                        attns[ti][:, si * 128:(si + 1) * 128],
                                        ident16)
                nc.vector.tensor_copy(
                    attnT[:, 0:ST, ti * 128:(ti + 1) * 128],
                    pT.rearrange("p (k j) -> p k j", k=ST))

                # out[t, r] = sum_s attn_u[t, s] kv[s, r]; normalize on eviction
                for rh in range(2):
                    p = psmm.tile([128, 512], F32)
                    for si in range(ST):
                        nc.tensor.matmul(p, attnT[:, si, ti * 128:(ti + 1) * 128],
                                         kv_sb[:, si, rh * 512:(rh + 1) * 512],
                                         start=(si == 0), stop=(si == ST - 1))
                    o = outpool.tile([128, 512], F16)
                    nc.scalar.mul(o, p, recips[ti])
                    nc.gpsimd.dma_start(
                        out=out[n, ti * 128:(ti + 1) * 128, rh * 512:(rh + 1) * 512],
                        in_=o)

    nc.compile()
    return nc


def make_in_maps(query, key_value_states, attention_mask, Wk):
    wkcb = np.ascontiguousarray(
        Wk.reshape(ET, 128, RT, 128).transpose(2, 1, 0, 3).reshape(RT, 128, E)
    ).astype(np.float16)
    in_maps = []
    for b in range(B):
        in_maps.append({
            "qT": np.ascontiguousarray(query[0, b].T).astype(np.float16),
            "kv": np.ascontiguousarray(key_value_states[:, b]).astype(np.float16),
            "kvT": np.ascontiguousarray(
                key_value_states[:, b].transpose(0, 2, 1)).astype(np.float16),
            "mask": np.ascontiguousarray(attention_mask[0, b]).astype(np.float32),
            "wkcb": wkcb,
        })
    return in_maps


def kernel(query, key_value_states, attention_mask, Wk, bk):
    query = np.asarray(query, dtype=np.float32)
    key_value_states = np.asarray(key_value_states, dtype=np.float32)
    attention_mask = np.asarray(attention_mask, dtype=np.float32)
    Wk = np.asarray(Wk, dtype=np.float32)
    del bk  # cancels inside the softmax (constant along the softmax axis)

    if not _NC_CACHE:
        _NC_CACHE.append(build_nc())
    nc = _NC_CACHE[0]

    in_maps = make_in_maps(query, key_value_states, attention_mask, Wk)
    res = run_bass_kernel_spmd(nc, in_maps, core_ids=list(range(B)))

    out = np.empty((N_CAND, B, T, R), dtype=np.float32)
    for b in range(B):
        out[:, b] = res.results[b]["out"].astype(np.float32)
    return out


# revision 8
# speedup vs baseline: 1.1129x; 1.0293x over previous
"""AlignmentAttention Trainium2 kernel (8 NeuronCores, pure data parallel over B).

Math: reference computes
    key    = einsum("nbsr,er->nbse", kv, Wk) + bk
    scores = einsum("bte,nbse->nbts", q, key) + mask
    out    = softmax(scores) @ kv
Because softmax is invariant to per-row constants, the bias term q@bk cancels,
and q @ (kv@Wk^T)^T == (q@Wk) @ kv^T.  So we project the *query* once per batch
element (qproj = q@Wk, shared across all N candidates) instead of projecting
keys N times, and drop bk entirely.

Sharding: one batch element b per core (B=8 == n_cores).  Per core:
    qprojT = (q_b @ Wk)^T              64 matmuls   (fp16 operands, f32 psum)
    per candidate n:
        scores  = qproj @ kvT_nb        32 matmuls
        softmax: DVE mask-add + rowmax, ACT fused exp+rowsum -> fp16 attn
        attn^T via fp16 PE transpose (1 cyc/row, fp16 psum)
        out_nb  = attn @ kv_nb          32 matmuls, 1/rowsum fused into the
                  fp16 eviction; host upcasts fp16 -> f32

Perf notes (from perfetto/NTFF analysis):
  - engine preambles delay all real work to ~9us; input DMA starts ~8.7us.
    A few fp16 identity transposes in the dead zone pull the PE p-state
    ramp (0.65 -> 1.2 -> 2.4 GHz) earlier so qproj runs at full clock.
  - chunked input DMAs (not one big load) keep Tile's dependency tracking
    slice-granular, so matmuls start as soon as their chunk lands.
  - kv/kvT pools are 3 deep: candidate n+2's loads overlap candidate n,
    hiding the buffer-reuse serialization that otherwise starves scores.
  - transpose+out matmuls are emitted per-ti so a late softmax(ti=3)
    cannot block ready out-matmuls behind it in the PE queue.
  - out is fp16 on the gpsimd queue: halves tail-store bytes and keeps
    DMA issue off the scalar engine (which runs exp + evictions).
"""
import contextlib
import os
import sys

import numpy as np

_TRN_REPO = "/opt/trn_rl_repo"
if _TRN_REPO not in sys.path and os.path.isdir(_TRN_REPO):
    sys.path.insert(0, _TRN_REPO)

# jax on the native neuron backend crashes; the axon PJRT proxy path needs the
# default platform selection.
if os.environ.get("JAX_PLATFORMS") == "cpu":
    os.environ["JAX_PLATFORMS"] = ""

import concourse.bacc as bacc
import concourse.tile as tile
from concourse import mybir
from concourse.masks import make_identity
from concourse.bass_utils import run_bass_kernel_spmd

F32 = mybir.dt.float32
F16 = mybir.dt.float16

N_CAND, B, T, S, E, R = 4, 8, 512, 512, 1024, 1024
TT, ST, ET, RT = T // 128, S // 128, E // 128, R // 128

_NC_CACHE = []


def build_nc():
    nc = bacc.Bacc(None, target_bir_lowering=False)
    qT = nc.declare_dram_parameter("qT", [E, T], F16, isOutput=False)
    kv = nc.declare_dram_parameter("kv", [N_CAND, S, R], F16, isOutput=False)
    kvT = nc.declare_dram_parameter("kvT", [N_CAND, R, S], F16, isOutput=False)
    mask = nc.declare_dram_parameter("mask", [T, S], F32, isOutput=False)
    wk = nc.declare_dram_parameter("wk", [E, R], F16, isOutput=False)
    out = nc.declare_dram_parameter("out", [N_CAND, T, R], F16, isOutput=True)

    with contextlib.ExitStack() as ctx:
        tc = ctx.enter_context(tile.TileContext(nc))
        singles = ctx.enter_context(tc.tile_pool(name="singles", bufs=1))
        kvpool = ctx.enter_context(tc.tile_pool(name="kvpool", bufs=3))
        kvtpool = ctx.enter_context(tc.tile_pool(name="kvtpool", bufs=3))
        scorepool = ctx.enter_context(tc.tile_pool(name="scorepool", bufs=3))
        attnpool = ctx.enter_context(tc.tile_pool(name="attnpool", bufs=3))
        attntpool = ctx.enter_context(tc.tile_pool(name="attntpool", bufs=2))
        outpool = ctx.enter_context(tc.tile_pool(name="outpool", bufs=3))
        smalls = ctx.enter_context(tc.tile_pool(name="smalls", bufs=10))
        psW = ctx.enter_context(tc.tile_pool(name="psW", bufs=1, space="PSUM"))
        psT = ctx.enter_context(tc.tile_pool(name="psT", bufs=2, space="PSUM"))
        psmm = ctx.enter_context(tc.tile_pool(name="psmm", bufs=5, space="PSUM"))

        ident16 = singles.tile([128, 128], F16)
        make_identity(nc, ident16)

        # Dead-zone warmup: the engine preamble blocks real work until ~9us
        # anyway; these free transposes start the PE p-state ramp early.
        wp = psW.tile([128, 512], F16, tag="warm")
        for k in range(8):
            nc.tensor.transpose(wp[:, (k % 4) * 128:(k % 4 + 1) * 128],
                                ident16, ident16)

        # persistent SBUF; interleave wk/qT chunks so qproj matmuls can chase
        # the arrivals (mm (r, e) needs only wk chunk e + qT chunk e).
        wk_sb = singles.tile([128, ET, R], F16)
        qT_sb = singles.tile([128, ET, T], F16)
        for e in range(ET):
            nc.sync.dma_start(out=wk_sb[:, e, :], in_=wk[e * 128:(e + 1) * 128, :])
            nc.sync.dma_start(out=qT_sb[:, e, :], in_=qT[e * 128:(e + 1) * 128, :])
        mask_sb = singles.tile([128, TT, S], F32)
        qprojT = singles.tile([128, RT, T], F16)

        # qprojT[r, t] = sum_e wk[e, r] * qT[e, t]
        for r in range(RT):
            p = psmm.tile([128, T], F32)
            for e in range(ET):
                nc.tensor.matmul(p, wk_sb[:, e, r * 128:(r + 1) * 128], qT_sb[:, e, :],
                                 start=(e == 0), stop=(e == ET - 1))
            nc.scalar.copy(qprojT[:, r, :], p)

        # mask is needed only at the first softmax; keep it off the critical
        # prologue path.
        for ti in range(TT):
            nc.sync.dma_start(out=mask_sb[:, ti, :], in_=mask[ti * 128:(ti + 1) * 128, :])

        for n in range(N_CAND):
            kvT_sb = kvtpool.tile([128, RT, S], F16)
            for ri in range(RT):
                nc.sync.dma_start(out=kvT_sb[:, ri, :],
                                  in_=kvT[n, ri * 128:(ri + 1) * 128, :])
            kv_sb = kvpool.tile([128, ST, R], F16)
            for si in range(ST):
                nc.sync.dma_start(out=kv_sb[:, si, :],
                                  in_=kv[n, si * 128:(si + 1) * 128, :])

            # scores[t, s] = sum_r qproj[t, r] * kv[s, r]
            score_ps = []
            for ti in range(TT):
                p = psmm.tile([128, S], F32)
                for ri in range(RT):
                    nc.tensor.matmul(p, qprojT[:, ri, ti * 128:(ti + 1) * 128],
                                     kvT_sb[:, ri, :],
                                     start=(ri == 0), stop=(ri == RT - 1))
                score_ps.append(p)

            # softmax, unnormalized: attn_u = exp(scores + mask - rowmax) in
            # fp16; keep 1/rowsum for the out-matmul eviction.
            attns = []
            recips = []
            for ti in range(TT):
                scores = scorepool.tile([128, S], F32)
                nc.vector.tensor_add(scores, score_ps[ti], mask_sb[:, ti, :])
                negmax = smalls.tile([128, 1], F32)
                nc.vector.tensor_reduce(negmax, scores, axis=mybir.AxisListType.X,
                                        op=mybir.AluOpType.max, negate=True)
                attn = attnpool.tile([128, S], F16)
                sumexp = smalls.tile([128, 1], F32)
                nc.scalar.activation(attn, scores, mybir.ActivationFunctionType.Exp,
                                     bias=negmax, scale=1.0, accum_out=sumexp)
                recip = smalls.tile([128, 1], F32)
                nc.vector.reciprocal(recip, sumexp)
                attns.append(attn)
                recips.append(recip)

            # per ti: fp16 PE transpose of attn, then immediately that ti's
            # out matmuls.
            attnT = attntpool.tile([128, ST, T], F16)
            for ti in range(TT):
                pT = psT.tile([128, 512], F16)
                for si in range(ST):
                    nc.tensor.transpose(pT[:, si * 128:(si + 1) * 128],
                                        attns[ti][:, si * 128:(si + 1) * 128],
                                        ident16)
                nc.vector.tensor_copy(
                    attnT[:, 0:ST, ti * 128:(ti + 1) * 128],
                    pT.rearrange("p (k j) -> p k j", k=ST))

                # out[t, r] = sum_s attn_u[t, s] kv[s, r]; normalize on eviction
                for rh in range(2):
                    p = psmm.tile([128, 512], F32)
                    for si in range(ST):
                        nc.tensor.matmul(p, attnT[:, si, ti * 128:(ti + 1) * 128],
                                         kv_sb[:, si, rh * 512:(rh + 1) * 512],
                                         start=(si == 0), stop=(si == ST - 1))
                    o = outpool.tile([128, 512], F16)
                    nc.scalar.mul(o, p, recips[ti])
                    nc.gpsimd.dma_start(
                        out=out[n, ti * 128:(ti + 1) * 128, rh * 512:(rh + 1) * 512],
                        in_=o)

    nc.compile()
    return nc


def make_in_maps(query, key_value_states, attention_mask, Wk):
    in_maps = []
    for b in range(B):
        in_maps.append({
            "qT": np.ascontiguousarray(query[0, b].T).astype(np.float16),
            "kv": np.ascontiguousarray(key_value_states[:, b]).astype(np.float16),
            "kvT": np.ascontiguousarray(
                key_value_states[:, b].transpose(0, 2, 1)).astype(np.float16),
            "mask": np.ascontiguousarray(attention_mask[0, b]).astype(np.float32),
            "wk": np.ascontiguousarray(Wk).astype(np.float16),
        })
    return in_maps


def kernel(query, key_value_states, attention_mask, Wk, bk):
    query = np.asarray(query, dtype=np.float32)
    key_value_states = np.asarray(key_value_states, dtype=np.float32)
    attention_mask = np.asarray(attention_mask, dtype=np.float32)
    Wk = np.asarray(Wk, dtype=np.float32)
    del bk  # cancels inside the softmax (constant along the softmax axis)

    if not _NC_CACHE:
        _NC_CACHE.append(build_nc())
    nc = _NC_CACHE[0]

    in_maps = make_in_maps(query, key_value_states, attention_mask, Wk)
    res = run_bass_kernel_spmd(nc, in_maps, core_ids=list(range(B)))

    out = np.empty((N_CAND, B, T, R), dtype=np.float32)
    for b in range(B):
        out[:, b] = res.results[b]["out"].astype(np.float32)
    return out


# revision 17
# speedup vs baseline: 1.1282x; 1.0138x over previous
"""AlignmentAttention Trainium2 kernel (8 NeuronCores, pure data parallel over B).

Math: reference computes
    key    = einsum("nbsr,er->nbse", kv, Wk) + bk
    scores = einsum("bte,nbse->nbts", q, key) + mask
    out    = softmax(scores) @ kv
Because softmax is invariant to per-row constants, the bias term q@bk cancels,
and q @ (kv@Wk^T)^T == (q@Wk) @ kv^T.  So we project the *query* once per batch
element (qproj = q@Wk, shared across all N candidates) instead of projecting
keys N times, and drop bk entirely.

Sharding: one batch element b per core (B=8 == n_cores).  Per core:
    qprojT = (q_b @ Wk)^T              64 matmuls   (fp16 operands, f32 psum)
    per candidate n:
        scores  = qproj @ kvT_nb        32 matmuls
        softmax: DVE mask-add + rowmax, ACT fused exp+rowsum -> fp16 attn
        attn^T via fp16 PE transpose (1 cyc/row, fp16 psum)
        out_nb  = attn @ kv_nb          32 matmuls, 1/rowsum fused into the
                  fp16 eviction; host upcasts fp16 -> f32

Perf notes (from perfetto/NTFF analysis):
  - engine preambles delay all real work to ~9us; input DMA starts ~8.7us.
    A few fp16 identity transposes in the dead zone pull the PE p-state
    ramp (0.65 -> 1.2 -> 2.4 GHz) earlier so qproj runs at full clock.
  - chunked input DMAs (not one big load) keep Tile's dependency tracking
    slice-granular, so matmuls start as soon as their chunk lands.
  - kv/kvT pools are 3 deep: candidate n+2's loads overlap candidate n,
    hiding the buffer-reuse serialization that otherwise starves scores.
  - transpose+out matmuls are emitted per-ti so a late softmax(ti=3)
    cannot block ready out-matmuls behind it in the PE queue.
  - out is fp16 on the gpsimd queue: halves tail-store bytes and keeps
    DMA issue off the scalar engine (which runs exp + evictions).
"""
import contextlib
import os
import sys

import numpy as np

_TRN_REPO = "/opt/trn_rl_repo"
if _TRN_REPO not in sys.path and os.path.isdir(_TRN_REPO):
    sys.path.insert(0, _TRN_REPO)

# jax on the native neuron backend crashes; the axon PJRT proxy path needs the
# default platform selection.
if os.environ.get("JAX_PLATFORMS") == "cpu":
    os.environ["JAX_PLATFORMS"] = ""

import concourse.bacc as bacc
import concourse.tile as tile
from concourse import mybir
from concourse.masks import make_identity
from concourse.bass_utils import run_bass_kernel_spmd

F32 = mybir.dt.float32
F16 = mybir.dt.float16

N_CAND, B, T, S, E, R = 4, 8, 512, 512, 1024, 1024
TT, ST, ET, RT = T // 128, S // 128, E // 128, R // 128

_NC_CACHE = []


def build_nc():
    nc = bacc.Bacc(None, target_bir_lowering=False)
    qT = nc.declare_dram_parameter("qT", [E, T], F16, isOutput=False)
    kv = nc.declare_dram_parameter("kv", [N_CAND, S, R], F16, isOutput=False)
    kvT = nc.declare_dram_parameter("kvT", [N_CAND, R, S], F16, isOutput=False)
    mask = nc.declare_dram_parameter("mask", [T, S], F32, isOutput=False)
    wk = nc.declare_dram_parameter("wk", [E, R], F16, isOutput=False)
    out = nc.declare_dram_parameter("out", [N_CAND, T, R], F16, isOutput=True)

    with contextlib.ExitStack() as ctx:
        tc = ctx.enter_context(tile.TileContext(nc))
        singles = ctx.enter_context(tc.tile_pool(name="singles", bufs=1))
        kvpool = ctx.enter_context(tc.tile_pool(name="kvpool", bufs=3))
        kvtpool = ctx.enter_context(tc.tile_pool(name="kvtpool", bufs=3))
        scorepool = ctx.enter_context(tc.tile_pool(name="scorepool", bufs=3))
        attnpool = ctx.enter_context(tc.tile_pool(name="attnpool", bufs=3))
        attntpool = ctx.enter_context(tc.tile_pool(name="attntpool", bufs=2))
        outpool = ctx.enter_context(tc.tile_pool(name="outpool", bufs=3))
        smalls = ctx.enter_context(tc.tile_pool(name="smalls", bufs=10))
        psT = ctx.enter_context(tc.tile_pool(name="psT", bufs=2, space="PSUM"))
        psmm = ctx.enter_context(tc.tile_pool(name="psmm", bufs=6, space="PSUM"))

        ident16 = singles.tile([128, 128], F16)
        make_identity(nc, ident16)

        # Dead-zone warmup: the engine preamble blocks real work until ~9us
        # anyway; these free transposes start the PE p-state ramp early.
        # Shares the psT slot ("pt") with the per-ti transpose tiles.
        wp = psT.tile([128, 512], F16, tag="pT")
        for k in range(8):
            nc.tensor.transpose(wp[:, (k % 4) * 128:(k % 4 + 1) * 128],
                                ident16, ident16)

        # persistent SBUF; interleave wk/qT chunks so qproj matmuls can chase
        # the arrivals (mm (r, e) needs only wk chunk e + qT chunk e).
        wk_sb = singles.tile([128, ET, R], F16)
        qT_sb = singles.tile([128, ET, T], F16)
        for e in range(ET):
            nc.sync.dma_start(out=wk_sb[:, e, :], in_=wk[e * 128:(e + 1) * 128, :])
            nc.sync.dma_start(out=qT_sb[:, e, :], in_=qT[e * 128:(e + 1) * 128, :])
        mask_sb = singles.tile([128, TT, S], F32)
        qprojT = singles.tile([128, RT, T], F16)

        # qprojT[r, t] = sum_e wk[e, r] * qT[e, t]
        # e-major in two passes of 6+2 r-blocks (6 live psum banks): the mm
        # stream chases the wk/qT chunk arrivals instead of being gated on
        # the last chunk for every r-block.
        for r0, r1 in ((0, 6), (6, RT)):
            ps = {r: psmm.tile([128, T], F32, name=f"qp{r}", tag="p")
                  for r in range(r0, r1)}
            for e in range(ET):
                for r in range(r0, r1):
                    nc.tensor.matmul(ps[r], wk_sb[:, e, r * 128:(r + 1) * 128],
                                     qT_sb[:, e, :],
                                     start=(e == 0), stop=(e == ET - 1))
            for r in range(r0, r1):
                nc.scalar.copy(qprojT[:, r, :], ps[r])

        # mask is needed only at the first softmax; keep it off the critical
        # prologue path.
        for ti in range(TT):
            nc.sync.dma_start(out=mask_sb[:, ti, :], in_=mask[ti * 128:(ti + 1) * 128, :])

        for n in range(N_CAND):
            kvT_sb = kvtpool.tile([128, RT, S], F16)
            for ri in range(RT):
                nc.sync.dma_start(out=kvT_sb[:, ri, :],
                                  in_=kvT[n, ri * 128:(ri + 1) * 128, :])
            kv_sb = kvpool.tile([128, ST, R], F16)
            for si in range(ST):
                nc.sync.dma_start(out=kv_sb[:, si, :],
                                  in_=kv[n, si * 128:(si + 1) * 128, :])

            # scores[t, s] = sum_r qproj[t, r] * kv[s, r]
            score_ps = []
            for ti in range(TT):
                p = psmm.tile([128, S], F32)
                for ri in range(RT):
                    nc.tensor.matmul(p, qprojT[:, ri, ti * 128:(ti + 1) * 128],
                                     kvT_sb[:, ri, :],
                                     start=(ri == 0), stop=(ri == RT - 1))
                score_ps.append(p)

            # softmax, unnormalized: attn_u = exp(scores + mask - rowmax) in
            # fp16.  Fused DVE pass: scoresN = -(scores + mask), negmax =
            # min(scoresN) = -rowmax; ACT then computes exp(-scoresN - rowmax)
            # with accumulated rowsum.  1/rowsum (DVE reciprocal) is only
            # needed at the out-matmul eviction, so it is emitted inside the
            # per-ti loop below to keep the DVE queue free for the attnT
            # copies (in-order engine queues: anything emitted before the
            # copy would delay the ready out-matmuls behind it).
            attns = []
            sumexps = []
            for ti in range(TT):
                scoresN = scorepool.tile([128, S], F32)
                negmax = smalls.tile([128, 1], F32)
                nc.vector.tensor_add(scoresN, score_ps[ti], mask_sb[:, ti, :])
                nc.vector.tensor_reduce(negmax, scoresN, axis=mybir.AxisListType.X,
                                        op=mybir.AluOpType.max, negate=True)
                attn = attnpool.tile([128, S], F16)
                sumexp = smalls.tile([128, 1], F32)
                nc.scalar.activation(attn, scoresN, mybir.ActivationFunctionType.Exp,
                                     bias=negmax, scale=1.0, accum_out=sumexp)
                attns.append(attn)
                sumexps.append(sumexp)

            # per ti: fp16 PE transpose of attn, then immediately that ti's
            # out matmuls.
            attnT = attntpool.tile([128, ST, T], F16)
            for ti in range(TT):
                pT = psT.tile([128, 512], F16)
                for si in range(ST):
                    nc.tensor.transpose(pT[:, si * 128:(si + 1) * 128],
                                        attns[ti][:, si * 128:(si + 1) * 128],
                                        ident16)
                nc.vector.tensor_copy(
                    attnT[:, 0:ST, ti * 128:(ti + 1) * 128],
                    pT.rearrange("p (k j) -> p k j", k=ST))
                recip = smalls.tile([128, 1], F32)
                nc.vector.reciprocal(recip, sumexps[ti])

                # out[t, r] = sum_s attn_u[t, s] kv[s, r]; normalize on eviction
                for rh in range(2):
                    p = psmm.tile([128, 512], F32)
                    for si in range(ST):
                        nc.tensor.matmul(p, attnT[:, si, ti * 128:(ti + 1) * 128],
                                         kv_sb[:, si, rh * 512:(rh + 1) * 512],
                                         start=(si == 0), stop=(si == ST - 1))
                    o = outpool.tile([128, 512], F16)
                    nc.scalar.mul(o, p, recip)
                    nc.gpsimd.dma_start(
                        out=out[n, ti * 128:(ti + 1) * 128, rh * 512:(rh + 1) * 512],
                        in_=o)

    nc.compile()
    return nc


def make_in_maps(query, key_value_states, attention_mask, Wk):
    in_maps = []
    for b in range(B):
        in_maps.append({
            "qT": np.ascontiguousarray(query[0, b].T).astype(np.float16),
            "kv": np.ascontiguousarray(key_value_states[:, b]).astype(np.float16),
            "kvT": np.ascontiguousarray(
                key_value_states[:, b].transpose(0, 2, 1)).astype(np.float16),
            "mask": np.ascontiguousarray(attention_mask[0, b]).astype(np.float32),
            "wk": np.ascontiguousarray(Wk).astype(np.float16),
        })
    return in_maps


def kernel(query, key_value_states, attention_mask, Wk, bk):
    query = np.asarray(query, dtype=np.float32)
    key_value_states = np.asarray(key_value_states, dtype=np.float32)
    attention_mask = np.asarray(attention_mask, dtype=np.float32)
    Wk = np.asarray(Wk, dtype=np.float32)
    del bk  # cancels inside the softmax (constant along the softmax axis)

    if not _NC_CACHE:
        _NC_CACHE.append(build_nc())
    nc = _NC_CACHE[0]

    in_maps = make_in_maps(query, key_value_states, attention_mask, Wk)
    res = run_bass_kernel_spmd(nc, in_maps, core_ids=list(range(B)))

    out = np.empty((N_CAND, B, T, R), dtype=np.float32)
    for b in range(B):
        out[:, b] = res.results[b]["out"].astype(np.float32)
    return out
